# revision 32
# baseline (speedup 1.0000x reference)
"""DSSIM loss kernel for Trainium2 (8 NeuronCores, data-parallel over batch).

Computes (1 - mean(SSIM map)) / 2 for output/target of shape [32, 3, 512, 512],
6x6 Gaussian window (sigma=1.5), VALID padding.

Math per channel-image, with m1 = conv(x), m2 = conv(y), Exx = conv(x^2),
Eyy = conv(y^2), Exy = conv(x*y):
  t    = m1*m2
  be   = m1^2 + m2^2
  n2'  = (Exy + c2/2) - t          # = sigma12 + c2/2
  d2f  = (Exx + Eyy + c2) - be     # = sigma1^2 + sigma2^2 + c2
  num' = (t + c1/2) * n2'
  den  = (be + c1) * d2f
  ssim = 4 * num' / den            # == (2 m1 m2 + c1)(2 s12 + c2) / (...)

Vertical conv on the TensorEngine as banded-matrix matmuls in bf16 (one
[128,123] stationary); 5 matmuls per row-chunk (m1, m2, P=x^2+y^2 pair, R=xy).
PSUM->SBUF copies on ScalarE cast to bf16 and fold x g3 (Horner prescale),
+c2, +c2/2 into Copy's scale/bias (additive constants pass through the
unit-sum horizontal conv).  Horizontal conv on the VectorEngine over one
flat contiguous row per chimg (garbage at 512-block seams lands in columns
the reduce never reads): taps are symmetric about index 3, so pair sums
(v1+v5), (v2+v4) (TensorTensor, 2x packed bf16) feed a 4-term Horner whose
first scale runs on ScalarE.  The SSIM formula front is bf16 TensorTensor
(2x); the tail is one fp32 scalar_tensor_tensor, reciprocal_approx_fast,
and a multiply; per-chunk row sums run on ScalarE via Copy+accum_out.  Each
core returns a [128, 60] partial-sum matrix; host reduces, applies the x4,
and forms the loss.
"""

import functools
import math
import time

import numpy as np

# Wall-clock of the most recent on-device SPMD execution (ns) -- includes
# host<->device staging through the PJRT tunnel.  After measure_hw_exec_ns()
# has run, this instead holds the NTFF-profiled on-device execution time.
LAST_EXEC_NS = None

B, C, H, W = 32, 3, 512, 512
N_CORES = 8
IMG_PER_CORE = B // N_CORES          # 4
CHIMG = IMG_PER_CORE * C             # 12 channel-images per core
WS = 6
SIGMA = 1.5
HO = H - WS + 1                      # 507
# Vertical conv chunk starts: each chunk reads input rows [s, s+128) and
# produces output rows [s, s+123). Chunk 3 contributes only its first 15
# rows (369..383); chunk 4 covers 384..506.
CHUNK_STARTS = (0, 123, 246, 369, 384)
CHUNK_USE = (123, 123, 123, 15, 123)
N_CHUNKS = len(CHUNK_STARTS)




def _gauss_taps():
    g = np.array(
        [math.exp(-((i - WS // 2) ** 2) / (2.0 * SIGMA**2)) for i in range(WS)],
        dtype=np.float64,
    )
    g = g / g.sum()
    return [float(v) for v in g]


def _band_matrix():
    """[128, 123] bf16: banded vertical-conv matrix (columns = output rows)."""
    import ml_dtypes

    g = _gauss_taps()
    band = np.zeros((128, 123), dtype=np.float32)
    for m in range(123):
        for j in range(WS):
            band[m + j, m] = g[j]
    return band.astype(ml_dtypes.bfloat16)


@functools.lru_cache(maxsize=4)
def _build_nc(c1: float, c2: float):
    import concourse.bass as bass
    import concourse.tile as tile
    from concourse import bacc, mybir

    f32 = mybir.dt.float32
    bf16 = mybir.dt.bfloat16
    Alu = mybir.AluOpType
    Act = mybir.ActivationFunctionType

    g = _gauss_taps()
    g2 = g[2]
    k0 = float(g[0] / g[1])
    k1 = float(g[1] / g[3])
    k2 = float(g[3] / g[2])

    nc = bacc.Bacc("TRN2", target_bir_lowering=False, debug=False,
                   num_devices=N_CORES)
    x_dram = nc.declare_dram_parameter("x", [CHIMG, H, W], bf16,
                                       isOutput=False)
    y_dram = nc.declare_dram_parameter("y", [CHIMG, H, W], bf16,
                                       isOutput=False)
    band_dram = nc.declare_dram_parameter("band", [128, 123], bf16,
                                          isOutput=False)
    out_dram = nc.declare_dram_parameter("partial", [128, CHIMG * N_CHUNKS],
                                         f32, isOutput=True)

    n_cols = CHIMG * N_CHUNKS

    with tile.TileContext(nc) as tc:
        with (
            tc.tile_pool(name="const", bufs=1) as const_pool,
            tc.tile_pool(name="inp", bufs=2) as inp_pool,
            tc.tile_pool(name="sig", bufs=2) as sig_pool,
            tc.tile_pool(name="vert", bufs=2) as vert_pool,
            tc.tile_pool(name="hor", bufs=1) as hor_pool,
            tc.tile_pool(name="form", bufs=1) as form_pool,
            tc.tile_pool(name="psum", bufs=2,
                         space=bass.MemorySpace.PSUM) as psum_pool,
        ):
            band_sb = const_pool.tile([128, 123], bf16)
            nc.sync.dma_start(band_sb[:], band_dram[:])

            acc_mat = const_pool.tile([128, n_cols], f32)
            nc.vector.memset(acc_mat[:], 0.0)

            # Flat per-chimg signal layout: column (sig*5 + chunk)*512 + j.
            # Horizontal-conv ops run over one contiguous row of FLAT
            # columns; values straddling a 512 block boundary are garbage
            # but land only in columns 507..511 of each block, which the
            # reduce never reads.  PAD columns feed only discarded outputs.
            FLAT = 4 * N_CHUNKS * W          # 10240
            PAD = 8
            SIG = N_CHUNKS * W               # 2560 per signal

            for i in range(CHIMG):
                # ---- vertical conv of all 5 chunks into v_pack ----
                v_pack = vert_pool.tile([123, FLAT + PAD], bf16,
                                        tag="vpack")
                nc.scalar.memzero(v_pack[:, FLAT:FLAT + PAD])
                for ci, r0 in enumerate(CHUNK_STARTS):
                    xt = inp_pool.tile([128, W], bf16, tag="xt")
                    nc.sync.dma_start(xt[:], x_dram[i, r0:r0 + 128, :])
                    yt = inp_pool.tile([128, W], bf16, tag="yt")
                    nc.sync.dma_start(yt[:], y_dram[i, r0:r0 + 128, :])

                    x2_t = sig_pool.tile([128, W], bf16, tag="x2")
                    nc.gpsimd.tensor_mul(x2_t[:], xt[:], xt[:])
                    y2_t = sig_pool.tile([128, W], bf16, tag="y2")
                    nc.scalar.square(y2_t[:], yt[:])
                    xy_t = sig_pool.tile([128, W], bf16, tag="xy")
                    nc.gpsimd.tensor_mul(xy_t[:], xt[:], yt[:])

                    ps_m1 = psum_pool.tile([123, W], f32, tag="psM1")
                    nc.tensor.matmul(ps_m1[:], band_sb[:], xt[:],
                                     start=True, stop=True)
                    ps_m2 = psum_pool.tile([123, W], f32, tag="psM2")
                    nc.tensor.matmul(ps_m2[:], band_sb[:], yt[:],
                                     start=True, stop=True)
                    ps_p = psum_pool.tile([123, W], f32, tag="psP")
                    nc.tensor.matmul(ps_p[:], band_sb[:], x2_t[:],
                                     start=True, stop=False)
                    nc.tensor.matmul(ps_p[:], band_sb[:], y2_t[:],
                                     start=False, stop=True)
                    ps_r = psum_pool.tile([123, W], f32, tag="psR")
                    nc.tensor.matmul(ps_r[:], band_sb[:], xy_t[:],
                                     start=True, stop=True)

                    # PSUM->SBUF on ScalarE: bf16 cast, x g3 Horner prescale,
                    # and the +c2 / +c2/2 biases (they pass through the
                    # unit-sum horizontal conv as bias/g3 -- see module doc).
                    cb = ci * W
                    nc.scalar.activation(v_pack[:, 0 * SIG + cb:0 * SIG + cb + W],
                                         ps_m1[:], Act.Copy, scale=g2)
                    nc.scalar.activation(v_pack[:, 1 * SIG + cb:1 * SIG + cb + W],
                                         ps_m2[:], Act.Copy, scale=g2)
                    nc.scalar.activation(v_pack[:, 2 * SIG + cb:2 * SIG + cb + W],
                                         ps_p[:], Act.Copy,
                                         bias=c2 * g2, scale=g2)
                    nc.scalar.activation(v_pack[:, 3 * SIG + cb:3 * SIG + cb + W],
                                         ps_r[:], Act.Copy,
                                         bias=0.5 * c2 * g2, scale=g2)

                # ---- horizontal conv (6 taps, symmetric about index 3) ----
                # Horner over pair sums, ordered so v_pack's last reader is
                # two ops before the chain end (frees the double-buffered
                # v_pack earlier for the next chimg's copies):
                #   u = (((v0*k0 + (v1+v5))*k1 + v3)*k2 + (v2+v4))
                # with k1 = g1/g3, k2 = g3/g2; true h = g2*u (folded into
                # the PSUM copies' scale).  The first scale runs on the
                # ScalarE; the two mid-chain scales split ~50/50 between
                # ScalarE and DVE to equalize engine load.
                SP = 6400
                h1 = hor_pool.tile([123, FLAT], bf16, tag="h1")
                nc.scalar.mul(h1[:], v_pack[:, 0:FLAT], k0)
                s1 = hor_pool.tile([123, FLAT], bf16, tag="s")
                nc.vector.tensor_add(s1[:], v_pack[:, 1:1 + FLAT],
                                     v_pack[:, 5:5 + FLAT])
                s2 = hor_pool.tile([123, FLAT], bf16, tag="s2")
                nc.vector.tensor_add(s2[:], v_pack[:, 2:2 + FLAT],
                                     v_pack[:, 4:4 + FLAT])
                h2 = hor_pool.tile([123, FLAT], bf16, tag="h2")
                nc.vector.tensor_add(h2[:], h1[:], s1[:])
                h3 = hor_pool.tile([123, FLAT], bf16, tag="h1")
                nc.scalar.mul(h3[:, 0:SP], h2[:, 0:SP], k1)
                nc.vector.tensor_scalar(h3[:, SP:FLAT], h2[:, SP:FLAT],
                                        k1, None, Alu.mult)
                h4 = hor_pool.tile([123, FLAT], bf16, tag="h2")
                nc.vector.tensor_add(h4[:], h3[:],
                                     v_pack[:, 3:3 + FLAT])
                h5 = hor_pool.tile([123, FLAT], bf16, tag="h1")
                nc.scalar.mul(h5[:, 0:SP], h4[:, 0:SP], k2)
                nc.vector.tensor_scalar(h5[:, SP:FLAT], h4[:, SP:FLAT],
                                        k2, None, Alu.mult)
                u = hor_pool.tile([123, FLAT], bf16, tag="h2")
                nc.vector.tensor_add(u[:], h5[:], s2[:])

                m1 = u[:, 0 * SIG:1 * SIG]
                m2 = u[:, 1 * SIG:2 * SIG]
                p2c = u[:, 2 * SIG:3 * SIG]
                r2c = u[:, 3 * SIG:4 * SIG]

                # ---- SSIM formula (on [123, SIG]; block cols 507..511 are
                # garbage and skipped by the reduce) ----
                # ScalarE squares are issued first; the DVE meanwhile runs
                # the t -> n2 -> num chain, which has no ScalarE inputs, so
                # `be` rarely has to wait.
                a2 = form_pool.tile([123, SIG], bf16, tag="a2")
                nc.scalar.square(a2[:], m1)
                b2 = form_pool.tile([123, SIG], bf16, tag="b2")
                nc.scalar.square(b2[:], m2)
                t_t = form_pool.tile([123, SIG], bf16, tag="t")
                nc.vector.tensor_mul(t_t[:], m1, m2)
                n2 = form_pool.tile([123, SIG], bf16, tag="n2")
                nc.vector.tensor_sub(n2[:], r2c, t_t[:])
                # c1/2 ~ 5e-5 is sub-ulp next to t ~ 0.25 in bf16; dropping
                # it keeps num a 2x packed TensorTensor (validated: shifts
                # the loss by ~1e-6 relative).
                num = form_pool.tile([123, SIG], bf16, tag="num")
                nc.vector.tensor_mul(num[:], t_t[:], n2[:])
                be = form_pool.tile([123, SIG], bf16, tag="t")
                nc.vector.tensor_add(be[:], a2[:], b2[:])
                d2f = form_pool.tile([123, SIG], bf16, tag="n2")
                nc.vector.tensor_sub(d2f[:], p2c, be[:])
                den = form_pool.tile([123, SIG], f32, tag="den")
                nc.vector.scalar_tensor_tensor(
                    den[:], be[:], c1, d2f[:], Alu.add, Alu.mult)

                # ssim = 4 * num / den; the x4 is applied host-side.
                rec = form_pool.tile([123, SIG], f32, tag="rec")
                nc.vector.reciprocal_approx_fast(rec[:], den[:])
                scr = form_pool.tile([123, SIG], f32, tag="den")
                nc.vector.tensor_mul(scr[:], num[:], rec[:])
                # Per-chunk row sums on ScalarE (Copy + accum_out) to keep
                # the reduce off the busy VectorE.
                for ci in range(N_CHUNKS):
                    nr = CHUNK_USE[ci]
                    col = i * N_CHUNKS + ci
                    dump = form_pool.tile([123, W], f32, tag="dump")
                    nc.scalar.activation(
                        dump[0:nr, 0:HO], scr[0:nr, ci * W:ci * W + HO],
                        Act.Copy, accum_out=acc_mat[0:nr, col:col + 1])

            nc.sync.dma_start(out_dram[:], acc_mat[:])

    nc.compile()
    return nc


def _prep_in_maps(output, target):
    import ml_dtypes

    x = np.asarray(output)
    y = np.asarray(target)
    assert x.shape == (B, C, H, W) and y.shape == (B, C, H, W)

    mx = float(x.max())
    mn = float(x.min())
    max_val = 255.0 if mx > 128.0 else 1.0
    min_val = -1.0 if mn < -0.5 else 0.0
    L = max_val - min_val
    c1 = float((0.01 * L) ** 2)
    c2 = float((0.03 * L) ** 2)

    xb = x.astype(ml_dtypes.bfloat16)
    yb = y.astype(ml_dtypes.bfloat16)
    band = _band_matrix()
    in_maps = []
    for core in range(N_CORES):
        sl = slice(core * IMG_PER_CORE, (core + 1) * IMG_PER_CORE)
        in_maps.append({
            "x": np.ascontiguousarray(xb[sl].reshape(CHIMG, H, W)),
            "y": np.ascontiguousarray(yb[sl].reshape(CHIMG, H, W)),
            "band": band,
        })
    return c1, c2, in_maps


def _reduce_results(res):
    total = 0.0
    for core in range(N_CORES):
        total += float(res.results[core]["partial"].astype(np.float64).sum())
    # device accumulates num'/den; ssim = 4 * num'/den
    mean_ssim = 4.0 * total / float(B * C * HO * HO)
    return np.asarray((1.0 - mean_ssim) / 2.0, dtype=np.float32)


def kernel(output: np.ndarray, target: np.ndarray) -> np.ndarray:
    from concourse.bass_utils import run_bass_kernel_spmd

    c1, c2, in_maps = _prep_in_maps(output, target)
    nc = _build_nc(c1, c2)

    global LAST_EXEC_NS
    t0 = time.perf_counter()
    res = run_bass_kernel_spmd(nc, in_maps, list(range(N_CORES)))
    wall_ns = int((time.perf_counter() - t0) * 1e9)
    if LAST_EXEC_NS is None:
        LAST_EXEC_NS = wall_ns
    return _reduce_results(res)


# ---------------------------------------------------------------------------
# NTFF-profiled hardware execution time.
#
# run_bass_kernel_spmd(trace=True) under axon needs the antenv.axon_hooks
# NTFF profile hook, which this image's antenv does not ship.  Recreate it
# in-process from the ctypes shim in trn_agent_boot and the injected
# libaxon_pjrt.so, then run one traced execution and report the
# neuron-profile execution window of the slowest profiled core.
# ---------------------------------------------------------------------------

def _install_ntff_hook():
    import sys
    import types

    try:
        from antenv.axon_hooks import get_axon_ntff_profile_hook  # noqa: F401
        return True  # real hook module present
    except ImportError:
        pass
    try:
        import antenv  # noqa: F401  # the image's real package must exist
        from trn_agent_boot.trn_boot import _ntff_profile_via_ctypes

        hook = _ntff_profile_via_ctypes("/opt/axon/libaxon_pjrt.so")
        if hook is None:
            return False
        mod = types.ModuleType("antenv.axon_hooks")
        _h = [hook]
        mod.set_axon_ntff_profile_hook = lambda h: _h.__setitem__(0, h)
        mod.get_axon_ntff_profile_hook = lambda: _h[0]
        sys.modules["antenv.axon_hooks"] = mod
        return True
    except Exception:
        return False


def measure_hw_exec_ns(output: np.ndarray, target: np.ndarray):
    """Run once with NTFF profiling; return (exec_time_ns, loss) or None."""
    if not _install_ntff_hook():
        return None
    import concourse.bass_utils as bu

    # Zero-egress container: skip the fish-bucket artifact upload.
    bu.upload_artifacts = lambda tmpdir: "local://" + tmpdir

    c1, c2, in_maps = _prep_in_maps(output, target)
    nc = _build_nc(c1, c2)
    try:
        res = bu.run_bass_kernel_spmd(nc, in_maps, list(range(N_CORES)),
                                      trace=True)
    except Exception:
        return None
    if res.exec_time_ns is None:
        return None
    global LAST_EXEC_NS
    LAST_EXEC_NS = int(res.exec_time_ns)
    return int(res.exec_time_ns), _reduce_results(res)


# revision 34
# speedup vs baseline: 1.0458x; 1.0458x over previous
"""DSSIM loss kernel for Trainium2 (8 NeuronCores, data-parallel over batch).

Computes (1 - mean(SSIM map)) / 2 for output/target of shape [32, 3, 512, 512],
6x6 Gaussian window (sigma=1.5), VALID padding.

Math per channel-image, with m1 = conv(x), m2 = conv(y), Exx = conv(x^2),
Eyy = conv(y^2), Exy = conv(x*y):
  t    = m1*m2
  be   = m1^2 + m2^2
  n2'  = (Exy + c2/2) - t          # = sigma12 + c2/2
  d2f  = (Exx + Eyy + c2) - be     # = sigma1^2 + sigma2^2 + c2
  num' = (t + c1/2) * n2'
  den  = (be + c1) * d2f
  ssim = 4 * num' / den            # == (2 m1 m2 + c1)(2 s12 + c2) / (...)

Vertical conv on the TensorEngine as banded-matrix matmuls in bf16 (one
[128,123] stationary); 5 matmuls per row-chunk (m1, m2, P=x^2+y^2 pair, R=xy).
PSUM->SBUF copies on ScalarE cast to bf16 and fold x g3 (Horner prescale),
+c2, +c2/2 into Copy's scale/bias (additive constants pass through the
unit-sum horizontal conv).  Horizontal conv on the VectorEngine over one
flat contiguous row per chimg (garbage at 512-block seams lands in columns
the reduce never reads): taps are symmetric about index 3, so pair sums
(v1+v5), (v2+v4) (TensorTensor, 2x packed bf16) feed a 4-term Horner whose
first scale runs on ScalarE.  The SSIM formula front is bf16 TensorTensor
(2x); the tail is one fp32 scalar_tensor_tensor, reciprocal_approx_fast,
and a multiply; per-chunk row sums run on ScalarE via Copy+accum_out.  Each
core returns a [128, 60] partial-sum matrix; host reduces, applies the x4,
and forms the loss.
"""

import functools
import math
import time

import numpy as np

# Wall-clock of the most recent on-device SPMD execution (ns) -- includes
# host<->device staging through the PJRT tunnel.  After measure_hw_exec_ns()
# has run, this instead holds the NTFF-profiled on-device execution time.
LAST_EXEC_NS = None

B, C, H, W = 32, 3, 512, 512
N_CORES = 8
IMG_PER_CORE = B // N_CORES          # 4
CHIMG = IMG_PER_CORE * C             # 12 channel-images per core
WS = 6
SIGMA = 1.5
HO = H - WS + 1                      # 507
# Vertical conv chunk starts: each chunk reads input rows [s, s+128) and
# produces output rows [s, s+123). Chunk 3 contributes only its first 15
# rows (369..383); chunk 4 covers 384..506.
CHUNK_STARTS = (0, 123, 246, 369, 384)
CHUNK_USE = (123, 123, 123, 15, 123)
N_CHUNKS = len(CHUNK_STARTS)




def _gauss_taps():
    g = np.array(
        [math.exp(-((i - WS // 2) ** 2) / (2.0 * SIGMA**2)) for i in range(WS)],
        dtype=np.float64,
    )
    g = g / g.sum()
    return [float(v) for v in g]


def _band_matrix():
    """[128, 123] bf16: banded vertical-conv matrix (columns = output rows)."""
    import ml_dtypes

    g = _gauss_taps()
    band = np.zeros((128, 123), dtype=np.float32)
    for m in range(123):
        for j in range(WS):
            band[m + j, m] = g[j]
    return band.astype(ml_dtypes.bfloat16)


@functools.lru_cache(maxsize=4)
def _build_nc(c1: float, c2: float):
    import concourse.bass as bass
    import concourse.tile as tile
    from concourse import bacc, mybir

    f32 = mybir.dt.float32
    bf16 = mybir.dt.bfloat16
    Alu = mybir.AluOpType
    Act = mybir.ActivationFunctionType

    g = _gauss_taps()
    g2 = g[2]
    k0 = float(g[0] / g[1])
    k1 = float(g[1] / g[3])
    k2 = float(g[3] / g[2])

    nc = bacc.Bacc("TRN2", target_bir_lowering=False, debug=False,
                   num_devices=N_CORES)
    x_dram = nc.declare_dram_parameter("x", [CHIMG, H, W], bf16,
                                       isOutput=False)
    y_dram = nc.declare_dram_parameter("y", [CHIMG, H, W], bf16,
                                       isOutput=False)
    band_dram = nc.declare_dram_parameter("band", [128, 123], bf16,
                                          isOutput=False)
    out_dram = nc.declare_dram_parameter("partial", [128, CHIMG * N_CHUNKS],
                                         f32, isOutput=True)

    n_cols = CHIMG * N_CHUNKS

    with tile.TileContext(nc) as tc:
        with (
            tc.tile_pool(name="const", bufs=1) as const_pool,
            tc.tile_pool(name="inp", bufs=2) as inp_pool,
            tc.tile_pool(name="sig", bufs=2) as sig_pool,
            tc.tile_pool(name="vert", bufs=2) as vert_pool,
            tc.tile_pool(name="hor", bufs=1) as hor_pool,
            tc.tile_pool(name="form", bufs=1) as form_pool,
            tc.tile_pool(name="psum", bufs=2,
                         space=bass.MemorySpace.PSUM) as psum_pool,
        ):
            band_sb = const_pool.tile([128, 123], bf16)
            nc.sync.dma_start(band_sb[:], band_dram[:])

            acc_mat = const_pool.tile([128, n_cols], f32)
            nc.vector.memset(acc_mat[:], 0.0)

            # Flat per-chimg signal layout: column (sig*5 + chunk)*512 + j.
            # Horizontal-conv ops run over one contiguous row of FLAT
            # columns; values straddling a 512 block boundary are garbage
            # but land only in columns 507..511 of each block, which the
            # reduce never reads.  PAD columns feed only discarded outputs.
            FLAT = 4 * N_CHUNKS * W          # 10240
            PAD = 8
            SIG = N_CHUNKS * W               # 2560 per signal

            for i in range(CHIMG):
                # ---- vertical conv of all 5 chunks into v_pack ----
                v_pack = vert_pool.tile([123, FLAT + PAD], bf16,
                                        tag="vpack")
                nc.scalar.memzero(v_pack[:, FLAT:FLAT + PAD])
                for ci, r0 in enumerate(CHUNK_STARTS):
                    xt = inp_pool.tile([128, W], bf16, tag="xt")
                    nc.sync.dma_start(xt[:], x_dram[i, r0:r0 + 128, :])
                    yt = inp_pool.tile([128, W], bf16, tag="yt")
                    nc.sync.dma_start(yt[:], y_dram[i, r0:r0 + 128, :])

                    x2_t = sig_pool.tile([128, W], bf16, tag="x2")
                    nc.scalar.square(x2_t[:], xt[:])
                    y2_t = sig_pool.tile([128, W], bf16, tag="y2")
                    nc.scalar.square(y2_t[:], yt[:])
                    xy_t = sig_pool.tile([128, W], bf16, tag="xy")
                    nc.gpsimd.tensor_mul(xy_t[:], xt[:], yt[:])

                    ps_m1 = psum_pool.tile([123, W], f32, tag="psM1")
                    nc.tensor.matmul(ps_m1[:], band_sb[:], xt[:],
                                     start=True, stop=True)
                    ps_m2 = psum_pool.tile([123, W], f32, tag="psM2")
                    nc.tensor.matmul(ps_m2[:], band_sb[:], yt[:],
                                     start=True, stop=True)
                    ps_p = psum_pool.tile([123, W], f32, tag="psP")
                    nc.tensor.matmul(ps_p[:], band_sb[:], x2_t[:],
                                     start=True, stop=False)
                    nc.tensor.matmul(ps_p[:], band_sb[:], y2_t[:],
                                     start=False, stop=True)
                    ps_r = psum_pool.tile([123, W], f32, tag="psR")
                    nc.tensor.matmul(ps_r[:], band_sb[:], xy_t[:],
                                     start=True, stop=True)

                    # PSUM->SBUF on ScalarE: bf16 cast, x g3 Horner prescale,
                    # and the +c2 / +c2/2 biases (they pass through the
                    # unit-sum horizontal conv as bias/g3 -- see module doc).
                    cb = ci * W
                    nc.scalar.activation(v_pack[:, 0 * SIG + cb:0 * SIG + cb + W],
                                         ps_m1[:], Act.Copy, scale=g2)
                    nc.scalar.activation(v_pack[:, 1 * SIG + cb:1 * SIG + cb + W],
                                         ps_m2[:], Act.Copy, scale=g2)
                    nc.scalar.activation(v_pack[:, 2 * SIG + cb:2 * SIG + cb + W],
                                         ps_p[:], Act.Copy,
                                         bias=c2 * g2, scale=g2)
                    nc.scalar.activation(v_pack[:, 3 * SIG + cb:3 * SIG + cb + W],
                                         ps_r[:], Act.Copy,
                                         bias=0.5 * c2 * g2, scale=g2)

                # ---- horizontal conv (6 taps, symmetric about index 3) ----
                # Horner over pair sums, ordered so v_pack's last reader is
                # two ops before the chain end (frees the double-buffered
                # v_pack earlier for the next chimg's copies):
                #   u = (((v0*k0 + (v1+v5))*k1 + v3)*k2 + (v2+v4))
                # with k1 = g1/g3, k2 = g3/g2; true h = g2*u (folded into
                # the PSUM copies' scale).  The first scale runs on the
                # ScalarE; the two mid-chain scales split ~50/50 between
                # ScalarE and DVE to equalize engine load.
                SP = 5120
                h1 = hor_pool.tile([123, FLAT], bf16, tag="h1")
                nc.scalar.mul(h1[:], v_pack[:, 0:FLAT], k0)
                s1 = hor_pool.tile([123, FLAT], bf16, tag="s")
                nc.vector.tensor_add(s1[:], v_pack[:, 1:1 + FLAT],
                                     v_pack[:, 5:5 + FLAT])
                s2 = hor_pool.tile([123, FLAT], bf16, tag="s2")
                nc.vector.tensor_add(s2[:], v_pack[:, 2:2 + FLAT],
                                     v_pack[:, 4:4 + FLAT])
                h2 = hor_pool.tile([123, FLAT], bf16, tag="h2")
                nc.vector.tensor_add(h2[:], h1[:], s1[:])
                h3 = hor_pool.tile([123, FLAT], bf16, tag="h1")
                nc.scalar.mul(h3[:, 0:SP], h2[:, 0:SP], k1)
                nc.vector.tensor_scalar(h3[:, SP:FLAT], h2[:, SP:FLAT],
                                        k1, None, Alu.mult)
                h4 = hor_pool.tile([123, FLAT], bf16, tag="h2")
                nc.vector.tensor_add(h4[:], h3[:],
                                     v_pack[:, 3:3 + FLAT])
                h5 = hor_pool.tile([123, FLAT], bf16, tag="h1")
                nc.scalar.mul(h5[:, 0:SP], h4[:, 0:SP], k2)
                nc.vector.tensor_scalar(h5[:, SP:FLAT], h4[:, SP:FLAT],
                                        k2, None, Alu.mult)
                u = hor_pool.tile([123, FLAT], bf16, tag="h2")
                nc.vector.tensor_add(u[:], h5[:], s2[:])

                m1 = u[:, 0 * SIG:1 * SIG]
                m2 = u[:, 1 * SIG:2 * SIG]
                p2c = u[:, 2 * SIG:3 * SIG]
                r2c = u[:, 3 * SIG:4 * SIG]

                # ---- SSIM formula (on [123, SIG]; block cols 507..511 are
                # garbage and skipped by the reduce) ----
                # ScalarE squares are issued first; the DVE meanwhile runs
                # the t -> n2 -> num chain, which has no ScalarE inputs, so
                # `be` rarely has to wait.
                a2 = form_pool.tile([123, SIG], bf16, tag="a2")
                nc.scalar.square(a2[:], m1)
                b2 = form_pool.tile([123, SIG], bf16, tag="b2")
                nc.scalar.square(b2[:], m2)
                t_t = form_pool.tile([123, SIG], bf16, tag="t")
                nc.vector.tensor_mul(t_t[:], m1, m2)
                n2 = form_pool.tile([123, SIG], bf16, tag="n2")
                nc.vector.tensor_sub(n2[:], r2c, t_t[:])
                # c1/2 ~ 5e-5 is sub-ulp next to t ~ 0.25 in bf16; dropping
                # it keeps num a 2x packed TensorTensor (validated: shifts
                # the loss by ~1e-6 relative).
                num = form_pool.tile([123, SIG], bf16, tag="num")
                nc.vector.tensor_mul(num[:], t_t[:], n2[:])
                be = form_pool.tile([123, SIG], bf16, tag="t")
                nc.vector.tensor_add(be[:], a2[:], b2[:])
                d2f = form_pool.tile([123, SIG], bf16, tag="n2")
                nc.vector.tensor_sub(d2f[:], p2c, be[:])
                den = form_pool.tile([123, SIG], f32, tag="den")
                nc.vector.scalar_tensor_tensor(
                    den[:], be[:], c1, d2f[:], Alu.add, Alu.mult)

                # ssim = 4 * num / den; the x4 is applied host-side.
                rec = form_pool.tile([123, SIG], f32, tag="rec")
                nc.vector.reciprocal_approx_fast(rec[:], den[:])
                scr = form_pool.tile([123, SIG], f32, tag="den")
                nc.vector.tensor_mul(scr[:], num[:], rec[:])
                # Per-chunk row sums on ScalarE (Copy + accum_out) to keep
                # the reduce off the busy VectorE.
                for ci in range(N_CHUNKS):
                    nr = CHUNK_USE[ci]
                    col = i * N_CHUNKS + ci
                    dump = form_pool.tile([123, W], f32, tag="dump")
                    nc.scalar.activation(
                        dump[0:nr, 0:HO], scr[0:nr, ci * W:ci * W + HO],
                        Act.Copy, accum_out=acc_mat[0:nr, col:col + 1])

            nc.sync.dma_start(out_dram[:], acc_mat[:])

    nc.compile()
    return nc


def _prep_in_maps(output, target):
    import ml_dtypes

    x = np.asarray(output)
    y = np.asarray(target)
    assert x.shape == (B, C, H, W) and y.shape == (B, C, H, W)

    mx = float(x.max())
    mn = float(x.min())
    max_val = 255.0 if mx > 128.0 else 1.0
    min_val = -1.0 if mn < -0.5 else 0.0
    L = max_val - min_val
    c1 = float((0.01 * L) ** 2)
    c2 = float((0.03 * L) ** 2)

    xb = x.astype(ml_dtypes.bfloat16)
    yb = y.astype(ml_dtypes.bfloat16)
    band = _band_matrix()
    in_maps = []
    for core in range(N_CORES):
        sl = slice(core * IMG_PER_CORE, (core + 1) * IMG_PER_CORE)
        in_maps.append({
            "x": np.ascontiguousarray(xb[sl].reshape(CHIMG, H, W)),
            "y": np.ascontiguousarray(yb[sl].reshape(CHIMG, H, W)),
            "band": band,
        })
    return c1, c2, in_maps


def _reduce_results(res):
    total = 0.0
    for core in range(N_CORES):
        total += float(res.results[core]["partial"].astype(np.float64).sum())
    # device accumulates num'/den; ssim = 4 * num'/den
    mean_ssim = 4.0 * total / float(B * C * HO * HO)
    return np.asarray((1.0 - mean_ssim) / 2.0, dtype=np.float32)


def kernel(output: np.ndarray, target: np.ndarray) -> np.ndarray:
    from concourse.bass_utils import run_bass_kernel_spmd

    c1, c2, in_maps = _prep_in_maps(output, target)
    nc = _build_nc(c1, c2)

    global LAST_EXEC_NS
    t0 = time.perf_counter()
    res = run_bass_kernel_spmd(nc, in_maps, list(range(N_CORES)))
    wall_ns = int((time.perf_counter() - t0) * 1e9)
    if LAST_EXEC_NS is None:
        LAST_EXEC_NS = wall_ns
    return _reduce_results(res)


# ---------------------------------------------------------------------------
# NTFF-profiled hardware execution time.
#
# run_bass_kernel_spmd(trace=True) under axon needs the antenv.axon_hooks
# NTFF profile hook, which this image's antenv does not ship.  Recreate it
# in-process from the ctypes shim in trn_agent_boot and the injected
# libaxon_pjrt.so, then run one traced execution and report the
# neuron-profile execution window of the slowest profiled core.
# ---------------------------------------------------------------------------

def _install_ntff_hook():
    import sys
    import types

    try:
        from antenv.axon_hooks import get_axon_ntff_profile_hook  # noqa: F401
        return True  # real hook module present
    except ImportError:
        pass
    try:
        import antenv  # noqa: F401  # the image's real package must exist
        from trn_agent_boot.trn_boot import _ntff_profile_via_ctypes

        hook = _ntff_profile_via_ctypes("/opt/axon/libaxon_pjrt.so")
        if hook is None:
            return False
        mod = types.ModuleType("antenv.axon_hooks")
        _h = [hook]
        mod.set_axon_ntff_profile_hook = lambda h: _h.__setitem__(0, h)
        mod.get_axon_ntff_profile_hook = lambda: _h[0]
        sys.modules["antenv.axon_hooks"] = mod
        return True
    except Exception:
        return False


def measure_hw_exec_ns(output: np.ndarray, target: np.ndarray):
    """Run once with NTFF profiling; return (exec_time_ns, loss) or None."""
    if not _install_ntff_hook():
        return None
    import concourse.bass_utils as bu

    # Zero-egress container: skip the fish-bucket artifact upload.
    bu.upload_artifacts = lambda tmpdir: "local://" + tmpdir

    c1, c2, in_maps = _prep_in_maps(output, target)
    nc = _build_nc(c1, c2)
    try:
        res = bu.run_bass_kernel_spmd(nc, in_maps, list(range(N_CORES)),
                                      trace=True)
    except Exception:
        return None
    if res.exec_time_ns is None:
        return None
    global LAST_EXEC_NS
    LAST_EXEC_NS = int(res.exec_time_ns)
    return int(res.exec_time_ns), _reduce_results(res)
